# revision 46
# baseline (speedup 1.0000x reference)
"""Trainium2 Bass kernel for nn_Discriminator (batched bilinear form).

scores[b] = features[b] . (summary[b] @ weight.T)   for b in [0, 131072)

Strategy: data-parallel over 8 NeuronCores (batch sharded, weight replicated).
Per core (16384 rows = 128 tiles of 128 rows, processed in pairs):
  - summary (cast-DMA fp32 -> bf16 in flight) AND features (fp32) both ride
    the single SWDGE queue: measured ~385 GB/s vs ~371 GB/s for the
    SWDGE+HWDGE split; wt + score outputs ride the HWDGE (sync) ring
  - PE transposes the S tiles in bf16 transpose mode -> PSUM
  - ACT copies PSUM -> SBUF (two halves; chunk-granular for the final pair)
  - PE: 4 accumulating full-rate bf16 matmuls per tile: ws = S @ W^T (PSUM
    fp32); bf16 moving operand streams 1 col/cycle (fp32r was 1.5 cyc/col)
  - DVE scalar_tensor_tensor fuses multiply+reduce: scores col = sum(F * ws)
  - score accumulators are transposed on PE + streamed out per 32 tiles,
    the last group split 26+6 so only a tiny flush trails the last tile
The next pair's transposes are emitted ahead of this pair's matmuls so the
PE fills the PSUM->SBUF copy wait with useful work; idle LDWEIGHTS pad the
PE duty cycle so the HAM clock gate never drops it to 1.2 GHz mid-drain.
F stays fp32 (feeds only the DVE reduce against the fp32 PSUM ws), so the
only precision loss is bf16 rounding of summary and weight (~2.3e-3 rel).
Steady state is HBM-bound: per core 64 MiB of compulsory reads at the
~358 GB/s HBM share = ~187 us floor; fast-core spans measure ~202 us
(= ~6.4 preamble + ~180 loads + ~6 drain + ~9.5 framework postamble).
"""

import numpy as np

B = 131072
H = 512
NCORES = 8
BC = B // NCORES      # rows per core
P = 128               # partitions
T = BC // P           # batch tiles per core (128)
CHUNKS = H // P       # k-chunks (4)
NB = 4                # batch tiles per DMA block (4 -> 1MiB F per dma_start;
                      # NB=8 measured identical queue throughput)
BUFS_BLOCKS = 10      # block double-buffering depth
# DMA self-pacing: unpaced, the HBM stack arbiter lets one core of each
# stack pair pull ~371 GB/s while starving its mate to ~300 (measured via
# the mate speeding to ~387 the moment its neighbor finished). Since the
# grade is the max over cores, cap each core's demand near the fair half
# of the 716 GB/s stack: both bulk streams ride the SWDGE queue, whose
# descriptor emission is paced by deterministic GpSimd memsets between
# blocks (timed nops are rejected by the Tile scheduler's simulator).
PACE_MEMSETS = 0      # memsets of [128,512] fp32 (~0.52us each) per block
PACE_FRAC_COLS = 160  # one extra partial memset: ~331 GB/s per-core demand
                      # (stack capacity measured ~686 GB/s for 2 cores; at
                      # >=345 each the arbiter starves one side again)
PACE_SKIP_HEAD = 4    # leading blocks issue unpaced (pipeline priming)
PACE_SKIP_TAIL = 2    # trailing blocks issue unpaced (clean drain)
BUFS_PST = 2          # PSUM transpose pool depth
BUFS_PSW = 3          # PSUM ws pool depth (3x2 banks + 2x1 = 8 banks)
BUFS_ST = 4
BUFS_SCR = 2
# streamed score-output groups (tile counts); extra small tail groups
# were tried and regress (each adds a PE+ACT+DMA chain to the drain)
OUT_GROUPS = (32, 32, 32, 32)
FIRST_BLOCKS = (2, 2)  # small leading blocks (tile counts)
LAST_BLOCKS = (2,)     # one small trailing block: halves the STT count
                       # stranded behind the final F arrival

_CACHE = {}


def _build():
    from concourse import bacc
    import concourse.mybir as mybir
    import concourse.tile as tile

    dt = mybir.dt
    nc = bacc.Bacc("TRN2", target_bir_lowering=False)

    feat = nc.dram_tensor("features", [BC, H], dt.float32, kind="ExternalInput")
    summ = nc.dram_tensor("summary", [BC, H], dt.float32, kind="ExternalInput")
    wt = nc.dram_tensor("wt", [H, H], dt.float32, kind="ExternalInput")  # weight.T
    ident_in = nc.dram_tensor("ident", [P, P], dt.float32, kind="ExternalInput")
    scores = nc.dram_tensor("scores", [BC], dt.float32, kind="ExternalOutput")

    # DRAM views
    feat_v = feat.ap().rearrange("(n p) h -> p n h", p=P)   # [128, T, 512]
    summ_v = summ.ap().rearrange("(n p) h -> p n h", p=P)
    wt_v = wt.ap().rearrange("(c p) h -> p c h", p=P)       # [128, 4, 512]
    scores_v = scores.ap().rearrange("(t p) -> t p", p=P)   # [T, 128]

    with tile.TileContext(nc) as tc:
        from contextlib import ExitStack
        with ExitStack() as ctx:
            singles = ctx.enter_context(tc.tile_pool(name="singles", bufs=1))
            blocks = ctx.enter_context(tc.tile_pool(name="blocks", bufs=BUFS_BLOCKS))
            stp = ctx.enter_context(tc.tile_pool(name="stp", bufs=BUFS_ST))
            scr = ctx.enter_context(tc.tile_pool(name="scr", bufs=BUFS_SCR))
            psT = ctx.enter_context(tc.tile_pool(name="psT", bufs=BUFS_PST, space="PSUM"))
            psW = ctx.enter_context(tc.tile_pool(name="psW", bufs=BUFS_PSW, space="PSUM"))

            # block schedule: small first blocks so compute starts early,
            # small last blocks so the final dependency chain is short,
            # NB-tile blocks in between.
            sched = []
            t0 = 0
            for size in FIRST_BLOCKS:
                if t0 < T:
                    sched.append((t0, size))
                    t0 += size
            t_end = T - sum(LAST_BLOCKS)
            while t0 < t_end:
                sz = min(NB, t_end - t0)
                sched.append((t0, sz))
                t0 += sz
            for size in LAST_BLOCKS:
                sched.append((t0, size))
                t0 += size
            assert t0 == T
            blk_start = {s: sz for s, sz in sched}

            n_blocks = len(sched)
            blk_idx = {s: i for i, (s, _) in enumerate(sched)}
            pace_scr = singles.tile([P, H], dt.float32, name="pace_scr")

            # The F halves of the last two blocks are issued at the very
            # end of the queue, so those blocks' S parts land ~4us before
            # the final F bytes and the PE transpose->copy->matmul chain
            # for the last pairs overlaps the remaining F stream; the
            # drain is then just the trailing DVE reduces plus the flush.
            defer_f = {s for s, _ in sched if s >= T - 4}
            deferred_f = []

            def load_block(t0, size):
                # S arrives pre-rounded to bf16 via SWDGE cast-DMA
                # (rounding commutes with the transpose, so results are
                # identical to rounding after the transpose). F rides the
                # same SWDGE queue so one paced producer gates all bulk
                # HBM demand.
                s_b = blocks.tile([P, size, H], dt.bfloat16,
                                  name="s_blk", tag="s_blk")
                f_b = blocks.tile([P, size, H], dt.float32,
                                  name="f_blk", tag="f_blk")
                bi = blk_idx[t0]
                if PACE_MEMSETS and (
                        PACE_SKIP_HEAD <= bi < n_blocks - PACE_SKIP_TAIL):
                    # WAW-chained memsets form a serial pace clock; the
                    # corner copies give this block's DMAs a dependency on
                    # it (the Tile scheduler would otherwise sink pure
                    # filler to the end of the program, see v11 post-mortem)
                    for _ in range(PACE_MEMSETS):
                        nc.gpsimd.memset(pace_scr[:], 0)
                    if PACE_FRAC_COLS:
                        nc.gpsimd.memset(pace_scr[:, 0:PACE_FRAC_COLS], 0)
                    nc.gpsimd.tensor_copy(s_b[0:1, 0:1, 0:2],
                                          pace_scr[0:1, 0:2])
                    nc.gpsimd.tensor_copy(f_b[0:1, 0:1, 0:2],
                                          pace_scr[0:1, 0:2])
                nc.gpsimd.dma_start(out=s_b[:], in_=summ_v[:, t0:t0 + size, :])
                if t0 in defer_f:
                    deferred_f.append((f_b, t0, size))
                else:
                    nc.gpsimd.dma_start(
                        out=f_b[:], in_=feat_v[:, t0:t0 + size, :])
                return s_b, f_b, t0

            ident = singles.tile([P, P], dt.float32)
            nc.sync.dma_start(out=ident[:], in_=ident_in[:])
            ident_r = singles.tile([P, P], dt.bfloat16)
            nc.scalar.copy(ident_r[:], ident[:])

            # HAM warmup: ~4us of dummy PE work while the first data blocks
            # are still in flight, so the real matmuls start at 2.4 GHz
            # instead of the cold 1.2 GHz.
            warm_ps = psT.tile([P, CHUNKS, P], dt.bfloat16, name="warm_ps",
                               tag="ps_t")
            for i in range(28):
                nc.tensor.transpose(
                    warm_ps[:, i % CHUNKS, :], ident_r[:], ident_r[:])


            assert sum(OUT_GROUPS) == T
            grp_bounds = []  # (start_tile, size) per score group
            gs = 0
            for sz in OUT_GROUPS:
                grp_bounds.append((gs, sz))
                gs += sz
            tile_grp = {}  # tile -> (group idx, col within group)
            for gi, (gs_, sz_) in enumerate(grp_bounds):
                for tt in range(sz_):
                    tile_grp[gs_ + tt] = (gi, tt)
            scores_accs = [
                singles.tile([P, sz], dt.float32,
                             name=f"sacc{g}", tag=f"sacc{g}")
                for g, (_, sz) in enumerate(grp_bounds)
            ]

            # tile index -> owning block start
            blk_of_tile = {}
            for t0_, sz_ in sched:
                for tt in range(t0_, t0_ + sz_):
                    blk_of_tile[tt] = t0_
            blk_cache = {}

            def get_block(t):
                t0_ = blk_of_tile[t]
                if t0_ not in blk_cache:
                    blk_cache[t0_] = load_block(t0_, blk_start[t0_])
                return blk_cache[t0_]

            # prime the first data block before wt on the DMA streams:
            # S0 gates the whole pipeline (~13us), wt chunk 0 isn't needed
            # until the first matmuls (~20us).
            get_block_prime = None  # placeholder for ordering clarity

            def emit_transposes(t):
                # transpose 2 S tiles: 8x [128,128] -> PSUM (fp32r mode)
                s_b, _, t0_ = get_block(t)
                nb = t - t0_
                ps_t = psT.tile([P, 2, CHUNKS, P], dt.bfloat16, name="ps_t",
                                tag="ps_t")
                for u in range(2):
                    for c in range(CHUNKS):
                        nc.tensor.transpose(
                            ps_t[:, u, c, :],
                            s_b[:, nb + u, c * P:(c + 1) * P],
                            ident_r[:],
                        )
                return ps_t

            G = T // 2  # tile pairs
            get_block(0)  # issue S0/F0 DMAs ahead of wt in the rings

            # wt on the F (sync) ring: keeps the S ring 1 MiB lighter so S
            # finishes ~3us before F and the last pair's transpose/matmul
            # chain overlaps the final F blocks.
            wt_sb = singles.tile([P, CHUNKS, H], dt.float32)
            wt_r = singles.tile([P, CHUNKS, H], dt.bfloat16)
            nc.sync.dma_start(out=wt_sb[:], in_=wt_v[:])
            nc.scalar.copy(wt_r[:], wt_sb[:])

            ps_t_cur = emit_transposes(0)
            for g in range(G):
                t = 2 * g

                # The PE idles ~1us per block waiting on DMA, which lets
                # the HAM activity monitor halve the PE clock in recurring
                # 3.4us windows (fatal if one lands on the final drain).
                # Standalone bf16 LDWEIGHTS are pure PE busy-work (no PSUM
                # write, next matmul reloads anyway) that keeps the duty
                # cycle high and the clock at 2.4 GHz. Sized so the PE
                # stays just below the DMA arrival rate (more makes the PE
                # the pipeline pacer); the last pair skips them so nothing
                # delays the final matmuls.
                if g < G - 1:
                    for _ in range(4):
                        nc.tensor.ldweights(ident_r[:])

                # copy PSUM -> SBUF in two halves so tile u=0's matmuls
                # can start while u=1 is still copying (chunk-granular
                # copies for the last pair were tried: with the deferred-F
                # tail they serialize into a ~5us MM->ACT ladder)
                st_r = stp.tile([P, 2, CHUNKS, P], dt.bfloat16)
                nc.scalar.copy(st_r[:, 0], ps_t_cur[:, 0])
                nc.scalar.copy(st_r[:, 1], ps_t_cur[:, 1])

                # next pair's transposes go ahead of this pair's matmuls in
                # the PE stream: they fill the copy-wait with useful work
                if g + 1 < G:
                    ps_t_cur = emit_transposes(t + 2)
                if t + 2 == T - 2:
                    # all tail blocks are touched now; queue their F parts
                    for f_d, t0_d, sz_d in deferred_f:
                        nc.gpsimd.dma_start(
                            out=f_d[:], in_=feat_v[:, t0_d:t0_d + sz_d, :])
                    deferred_f.clear()

                # ws[b, h] = sum_k S[b, k] * W[h, k] : 4 accumulating matmuls/tile
                ps_w = psW.tile([P, 2, H], dt.float32)
                for u in range(2):
                    for c in range(CHUNKS):
                        nc.tensor.matmul(
                            ps_w[:, u, :],
                            st_r[:, u, c, :],
                            wt_r[:, c, :],
                            start=(c == 0),
                            stop=(c == CHUNKS - 1),
                        )

                # scores[:, t] = sum_h F * ws  (fused multiply+reduce on DVE)
                _, f_b, t0_ = get_block(t)
                nb = t - t0_
                for u in range(2):
                    gi, col = tile_grp[t + u]
                    mm_scr = scr.tile([P, H], dt.float32)
                    nc.vector.scalar_tensor_tensor(
                        out=mm_scr[:],
                        in0=f_b[:, nb + u, :],
                        scalar=1.0,
                        in1=ps_w[:, u, :],
                        op0=mybir.AluOpType.mult,
                        op1=mybir.AluOpType.mult,
                        accum_out=scores_accs[gi][:, col:col + 1],
                    )

                # stream completed score groups out:
                # scores_acc[p, tl] -> scores[(g0+tl)*128 + p]
                # the final group is flushed in two pieces (cols 0-29 as
                # soon as they are reduced, cols 30-31 after the last STT)
                # so only a tiny transpose+copy+DMA trails the last tile.
                gi, col = tile_grp[t + 1]
                g0, gsz = grp_bounds[gi]
                last_gi = len(grp_bounds) - 1

                def flush(gi, g0, c0, c1):
                    n = c1 - c0
                    ps_fin = psW.tile([n, P], dt.float32, tag="ps_w")
                    nc.tensor.transpose(
                        ps_fin[:], scores_accs[gi][:, c0:c1], ident[:])
                    out_sb = scr.tile([n, P], dt.float32, tag="out_sb")
                    nc.scalar.copy(out_sb[:], ps_fin[:])
                    nc.sync.dma_start(
                        out=scores_v[g0 + c0:g0 + c1, :], in_=out_sb[:])

                if gi < last_gi:
                    if col == gsz - 1:
                        flush(gi, g0, 0, gsz)
                else:
                    # flush well before the end: its PE transpose + ACT
                    # copy otherwise wedge into the last pairs' ACT queue
                    if col == gsz - 7:
                        flush(gi, g0, 0, gsz - 6)
                    elif col == gsz - 1:
                        flush(gi, g0, gsz - 6, gsz)

    nc.finalize()
    return nc


def _get_nc():
    if "nc" not in _CACHE:
        _CACHE["nc"] = _build()
    return _CACHE["nc"]


def kernel(features, summary, weight):
    from concourse.bass_utils import run_bass_kernel_spmd

    features = np.ascontiguousarray(np.asarray(features, dtype=np.float32))
    summary = np.ascontiguousarray(np.asarray(summary, dtype=np.float32))
    weight = np.asarray(weight, dtype=np.float32)
    wt = np.ascontiguousarray(weight.T)

    ident = np.eye(P, dtype=np.float32)
    nc = _get_nc()
    in_maps = [
        {
            "features": features[i * BC:(i + 1) * BC],
            "summary": summary[i * BC:(i + 1) * BC],
            "wt": wt,
            "ident": ident,
        }
        for i in range(NCORES)
    ]
    res = run_bass_kernel_spmd(nc, in_maps, core_ids=list(range(NCORES)))
    return np.concatenate([r["scores"] for r in res.results])


if __name__ == "__main__":
    rng = np.random.default_rng(0)
    f = rng.standard_normal((B, H), dtype=np.float32)
    s = rng.standard_normal((B, H), dtype=np.float32)
    w = (rng.random((H, H), dtype=np.float32) - 0.5) * (2.0 / np.sqrt(H))
    got = kernel(f, s, w)
    want = ((s @ w.T) * f).sum(-1)
    err = np.abs(got - want)
    print("absmax-rel:", err.max() / np.abs(want).max())



# revision 47
# speedup vs baseline: 1.0392x; 1.0392x over previous
"""Trainium2 Bass kernel for nn_Discriminator (batched bilinear form).

scores[b] = features[b] . (summary[b] @ weight.T)   for b in [0, 131072)

Strategy: data-parallel over 8 NeuronCores (batch sharded, weight replicated).
Per core (16384 rows = 128 tiles of 128 rows, processed in pairs):
  - summary (cast-DMA fp32 -> bf16 in flight) AND features (fp32) both ride
    the single SWDGE queue: measured ~385 GB/s vs ~371 GB/s for the
    SWDGE+HWDGE split; wt + score outputs ride the HWDGE (sync) ring
  - PE transposes the S tiles in bf16 transpose mode -> PSUM
  - ACT copies PSUM -> SBUF (two halves; chunk-granular for the final pair)
  - PE: 4 accumulating full-rate bf16 matmuls per tile: ws = S @ W^T (PSUM
    fp32); bf16 moving operand streams 1 col/cycle (fp32r was 1.5 cyc/col)
  - DVE scalar_tensor_tensor fuses multiply+reduce: scores col = sum(F * ws)
  - score accumulators are transposed on PE + streamed out per 32 tiles,
    the last group split 26+6 so only a tiny flush trails the last tile
The next pair's transposes are emitted ahead of this pair's matmuls so the
PE fills the PSUM->SBUF copy wait with useful work; idle LDWEIGHTS pad the
PE duty cycle so the HAM clock gate never drops it to 1.2 GHz mid-drain.
F stays fp32 (feeds only the DVE reduce against the fp32 PSUM ws), so the
only precision loss is bf16 rounding of summary and weight (~2.3e-3 rel).
Steady state is HBM-bound: per core 64 MiB of compulsory reads at the
~358 GB/s HBM share = ~187 us floor; fast-core spans measure ~202 us
(= ~6.4 preamble + ~180 loads + ~6 drain + ~9.5 framework postamble).
"""

import numpy as np

B = 131072
H = 512
NCORES = 8
BC = B // NCORES      # rows per core
P = 128               # partitions
T = BC // P           # batch tiles per core (128)
CHUNKS = H // P       # k-chunks (4)
NB = 4                # batch tiles per DMA block (4 -> 1MiB F per dma_start;
                      # NB=8 measured identical queue throughput)
BUFS_BLOCKS = 10      # block double-buffering depth
# DMA self-pacing: unpaced, the HBM stack arbiter lets one core of each
# stack pair pull ~371 GB/s while starving its mate to ~300 (measured via
# the mate speeding to ~387 the moment its neighbor finished). Since the
# grade is the max over cores, cap each core's demand near the fair half
# of the 716 GB/s stack: both bulk streams ride the SWDGE queue, whose
# descriptor emission is paced by deterministic GpSimd memsets between
# blocks (timed nops are rejected by the Tile scheduler's simulator).
PACE_MEMSETS = 0      # memsets of [128,512] fp32 (~0.52us each) per block
PACE_FRAC_COLS = 160  # one extra partial memset: ~331 GB/s per-core demand
                      # (stack capacity measured ~686 GB/s for 2 cores; at
                      # >=345 each the arbiter starves one side again)
PACE_SKIP_HEAD = 4    # leading blocks issue unpaced (pipeline priming)
PACE_SKIP_TAIL = 2    # trailing blocks issue unpaced (clean drain)
BUFS_PST = 2          # PSUM transpose pool depth
BUFS_PSW = 3          # PSUM ws pool depth (3x2 banks + 2x1 = 8 banks)
BUFS_ST = 4
BUFS_SCR = 2
# streamed score-output groups (tile counts); extra small tail groups
# were tried and regress (each adds a PE+ACT+DMA chain to the drain)
OUT_GROUPS = (32, 32, 32, 32)
FIRST_BLOCKS = (2, 2)  # small leading blocks (tile counts)
LAST_BLOCKS = (2,)     # one small trailing block: halves the STT count
                       # stranded behind the final F arrival

_CACHE = {}


def _build():
    from concourse import bacc
    import concourse.mybir as mybir
    import concourse.tile as tile

    dt = mybir.dt
    nc = bacc.Bacc("TRN2", target_bir_lowering=False)

    feat = nc.dram_tensor("features", [BC, H], dt.float32, kind="ExternalInput")
    summ = nc.dram_tensor("summary", [BC, H], dt.float32, kind="ExternalInput")
    wt = nc.dram_tensor("wt", [H, H], dt.float32, kind="ExternalInput")  # weight.T
    ident_in = nc.dram_tensor("ident", [P, P], dt.float32, kind="ExternalInput")
    scores = nc.dram_tensor("scores", [BC], dt.float32, kind="ExternalOutput")

    # DRAM views
    feat_v = feat.ap().rearrange("(n p) h -> p n h", p=P)   # [128, T, 512]
    summ_v = summ.ap().rearrange("(n p) h -> p n h", p=P)
    wt_v = wt.ap().rearrange("(c p) h -> p c h", p=P)       # [128, 4, 512]
    scores_v = scores.ap().rearrange("(t p) -> t p", p=P)   # [T, 128]

    with tile.TileContext(nc) as tc:
        from contextlib import ExitStack
        with ExitStack() as ctx:
            singles = ctx.enter_context(tc.tile_pool(name="singles", bufs=1))
            blocks = ctx.enter_context(tc.tile_pool(name="blocks", bufs=BUFS_BLOCKS))
            stp = ctx.enter_context(tc.tile_pool(name="stp", bufs=BUFS_ST))
            scr = ctx.enter_context(tc.tile_pool(name="scr", bufs=BUFS_SCR))
            psT = ctx.enter_context(tc.tile_pool(name="psT", bufs=BUFS_PST, space="PSUM"))
            psW = ctx.enter_context(tc.tile_pool(name="psW", bufs=BUFS_PSW, space="PSUM"))

            # block schedule: small first blocks so compute starts early,
            # small last blocks so the final dependency chain is short,
            # NB-tile blocks in between.
            sched = []
            t0 = 0
            for size in FIRST_BLOCKS:
                if t0 < T:
                    sched.append((t0, size))
                    t0 += size
            t_end = T - sum(LAST_BLOCKS)
            while t0 < t_end:
                sz = min(NB, t_end - t0)
                sched.append((t0, sz))
                t0 += sz
            for size in LAST_BLOCKS:
                sched.append((t0, size))
                t0 += size
            assert t0 == T
            blk_start = {s: sz for s, sz in sched}

            n_blocks = len(sched)
            blk_idx = {s: i for i, (s, _) in enumerate(sched)}
            pace_scr = singles.tile([P, H], dt.float32, name="pace_scr")

            # The F halves of the last two blocks are issued at the very
            # end of the queue, so those blocks' S parts land ~4us before
            # the final F bytes and the PE transpose->copy->matmul chain
            # for the last pairs overlaps the remaining F stream; the
            # drain is then just the trailing DVE reduces plus the flush.
            defer_f = {s for s, _ in sched if s >= T - 4}
            deferred_f = []

            def load_block(t0, size):
                # S arrives pre-rounded to bf16 via SWDGE cast-DMA
                # (rounding commutes with the transpose, so results are
                # identical to rounding after the transpose). F rides the
                # same SWDGE queue so one paced producer gates all bulk
                # HBM demand.
                s_b = blocks.tile([P, size, H], dt.bfloat16,
                                  name="s_blk", tag="s_blk")
                f_b = blocks.tile([P, size, H], dt.float32,
                                  name="f_blk", tag="f_blk")
                bi = blk_idx[t0]
                if PACE_MEMSETS and (
                        PACE_SKIP_HEAD <= bi < n_blocks - PACE_SKIP_TAIL):
                    # WAW-chained memsets form a serial pace clock; the
                    # corner copies give this block's DMAs a dependency on
                    # it (the Tile scheduler would otherwise sink pure
                    # filler to the end of the program, see v11 post-mortem)
                    for _ in range(PACE_MEMSETS):
                        nc.gpsimd.memset(pace_scr[:], 0)
                    if PACE_FRAC_COLS:
                        nc.gpsimd.memset(pace_scr[:, 0:PACE_FRAC_COLS], 0)
                    nc.gpsimd.tensor_copy(s_b[0:1, 0:1, 0:2],
                                          pace_scr[0:1, 0:2])
                    nc.gpsimd.tensor_copy(f_b[0:1, 0:1, 0:2],
                                          pace_scr[0:1, 0:2])
                nc.gpsimd.dma_start(out=s_b[:], in_=summ_v[:, t0:t0 + size, :])
                if t0 in defer_f:
                    deferred_f.append((f_b, t0, size))
                else:
                    nc.gpsimd.dma_start(
                        out=f_b[:], in_=feat_v[:, t0:t0 + size, :])
                return s_b, f_b, t0

            ident = singles.tile([P, P], dt.float32)
            nc.sync.dma_start(out=ident[:], in_=ident_in[:])
            ident_r = singles.tile([P, P], dt.bfloat16)
            nc.scalar.copy(ident_r[:], ident[:])

            # HAM warmup: ~4us of dummy PE work while the first data blocks
            # are still in flight, so the real matmuls start at 2.4 GHz
            # instead of the cold 1.2 GHz.
            warm_ps = psT.tile([P, CHUNKS, P], dt.bfloat16, name="warm_ps",
                               tag="ps_t")
            for i in range(28):
                nc.tensor.transpose(
                    warm_ps[:, i % CHUNKS, :], ident_r[:], ident_r[:])


            assert sum(OUT_GROUPS) == T
            grp_bounds = []  # (start_tile, size) per score group
            gs = 0
            for sz in OUT_GROUPS:
                grp_bounds.append((gs, sz))
                gs += sz
            tile_grp = {}  # tile -> (group idx, col within group)
            for gi, (gs_, sz_) in enumerate(grp_bounds):
                for tt in range(sz_):
                    tile_grp[gs_ + tt] = (gi, tt)
            scores_accs = [
                singles.tile([P, sz], dt.float32,
                             name=f"sacc{g}", tag=f"sacc{g}")
                for g, (_, sz) in enumerate(grp_bounds)
            ]

            # tile index -> owning block start
            blk_of_tile = {}
            for t0_, sz_ in sched:
                for tt in range(t0_, t0_ + sz_):
                    blk_of_tile[tt] = t0_
            blk_cache = {}

            def get_block(t):
                t0_ = blk_of_tile[t]
                if t0_ not in blk_cache:
                    blk_cache[t0_] = load_block(t0_, blk_start[t0_])
                return blk_cache[t0_]

            # prime the first data block before wt on the DMA streams:
            # S0 gates the whole pipeline (~13us), wt chunk 0 isn't needed
            # until the first matmuls (~20us).
            get_block_prime = None  # placeholder for ordering clarity

            def emit_transposes(t):
                # transpose 2 S tiles: 8x [128,128] -> PSUM (fp32r mode)
                s_b, _, t0_ = get_block(t)
                nb = t - t0_
                ps_t = psT.tile([P, 2, CHUNKS, P], dt.bfloat16, name="ps_t",
                                tag="ps_t")
                for u in range(2):
                    for c in range(CHUNKS):
                        nc.tensor.transpose(
                            ps_t[:, u, c, :],
                            s_b[:, nb + u, c * P:(c + 1) * P],
                            ident_r[:],
                        )
                return ps_t

            G = T // 2  # tile pairs
            get_block(0)  # issue S0/F0 DMAs ahead of wt in the rings

            # wt on the F (sync) ring: keeps the S ring 1 MiB lighter so S
            # finishes ~3us before F and the last pair's transpose/matmul
            # chain overlaps the final F blocks.
            wt_sb = singles.tile([P, CHUNKS, H], dt.float32)
            wt_r = singles.tile([P, CHUNKS, H], dt.bfloat16)
            nc.sync.dma_start(out=wt_sb[:], in_=wt_v[:])
            nc.scalar.copy(wt_r[:], wt_sb[:])

            ps_t_cur = emit_transposes(0)
            for g in range(G):
                t = 2 * g

                # The PE idles ~1-2us waiting for the last S blocks, which
                # trips the HAM activity monitor and halves the PE clock
                # for the whole drain. Standalone bf16 LDWEIGHTS are pure
                # PE busy-work (no PSUM write, next matmul reloads anyway)
                # that keeps the clock at 2.4 GHz through the tail. Tail
                # pairs only: spreading fillers across every pair makes
                # the PE the pipeline pacer whenever the chip drops to the
                # P0 power state (~2.0 GHz), costing +20us on every core.
                if G - 2 <= g < G - 1:
                    for _ in range(8):
                        nc.tensor.ldweights(ident_r[:])

                # copy PSUM -> SBUF in two halves so tile u=0's matmuls
                # can start while u=1 is still copying (chunk-granular
                # copies for the last pair were tried: with the deferred-F
                # tail they serialize into a ~5us MM->ACT ladder)
                st_r = stp.tile([P, 2, CHUNKS, P], dt.bfloat16)
                nc.scalar.copy(st_r[:, 0], ps_t_cur[:, 0])
                nc.scalar.copy(st_r[:, 1], ps_t_cur[:, 1])

                # next pair's transposes go ahead of this pair's matmuls in
                # the PE stream: they fill the copy-wait with useful work
                if g + 1 < G:
                    ps_t_cur = emit_transposes(t + 2)
                if t + 2 == T - 2:
                    # all tail blocks are touched now; queue their F parts
                    for f_d, t0_d, sz_d in deferred_f:
                        nc.gpsimd.dma_start(
                            out=f_d[:], in_=feat_v[:, t0_d:t0_d + sz_d, :])
                    deferred_f.clear()

                # ws[b, h] = sum_k S[b, k] * W[h, k] : 4 accumulating matmuls/tile
                ps_w = psW.tile([P, 2, H], dt.float32)
                for u in range(2):
                    for c in range(CHUNKS):
                        nc.tensor.matmul(
                            ps_w[:, u, :],
                            st_r[:, u, c, :],
                            wt_r[:, c, :],
                            start=(c == 0),
                            stop=(c == CHUNKS - 1),
                        )

                # scores[:, t] = sum_h F * ws  (fused multiply+reduce on DVE)
                _, f_b, t0_ = get_block(t)
                nb = t - t0_
                for u in range(2):
                    gi, col = tile_grp[t + u]
                    mm_scr = scr.tile([P, H], dt.float32)
                    nc.vector.scalar_tensor_tensor(
                        out=mm_scr[:],
                        in0=f_b[:, nb + u, :],
                        scalar=1.0,
                        in1=ps_w[:, u, :],
                        op0=mybir.AluOpType.mult,
                        op1=mybir.AluOpType.mult,
                        accum_out=scores_accs[gi][:, col:col + 1],
                    )

                # stream completed score groups out:
                # scores_acc[p, tl] -> scores[(g0+tl)*128 + p]
                # the final group is flushed in two pieces (cols 0-29 as
                # soon as they are reduced, cols 30-31 after the last STT)
                # so only a tiny transpose+copy+DMA trails the last tile.
                gi, col = tile_grp[t + 1]
                g0, gsz = grp_bounds[gi]
                last_gi = len(grp_bounds) - 1

                def flush(gi, g0, c0, c1):
                    n = c1 - c0
                    ps_fin = psW.tile([n, P], dt.float32, tag="ps_w")
                    nc.tensor.transpose(
                        ps_fin[:], scores_accs[gi][:, c0:c1], ident[:])
                    out_sb = scr.tile([n, P], dt.float32, tag="out_sb")
                    nc.scalar.copy(out_sb[:], ps_fin[:])
                    nc.sync.dma_start(
                        out=scores_v[g0 + c0:g0 + c1, :], in_=out_sb[:])

                if gi < last_gi:
                    if col == gsz - 1:
                        flush(gi, g0, 0, gsz)
                else:
                    # flush well before the end: its PE transpose + ACT
                    # copy otherwise wedge into the last pairs' ACT queue
                    if col == gsz - 7:
                        flush(gi, g0, 0, gsz - 6)
                    elif col == gsz - 1:
                        flush(gi, g0, gsz - 6, gsz)

    nc.finalize()
    return nc


def _get_nc():
    if "nc" not in _CACHE:
        _CACHE["nc"] = _build()
    return _CACHE["nc"]


def kernel(features, summary, weight):
    from concourse.bass_utils import run_bass_kernel_spmd

    features = np.ascontiguousarray(np.asarray(features, dtype=np.float32))
    summary = np.ascontiguousarray(np.asarray(summary, dtype=np.float32))
    weight = np.asarray(weight, dtype=np.float32)
    wt = np.ascontiguousarray(weight.T)

    ident = np.eye(P, dtype=np.float32)
    nc = _get_nc()
    in_maps = [
        {
            "features": features[i * BC:(i + 1) * BC],
            "summary": summary[i * BC:(i + 1) * BC],
            "wt": wt,
            "ident": ident,
        }
        for i in range(NCORES)
    ]
    res = run_bass_kernel_spmd(nc, in_maps, core_ids=list(range(NCORES)))
    return np.concatenate([r["scores"] for r in res.results])


if __name__ == "__main__":
    rng = np.random.default_rng(0)
    f = rng.standard_normal((B, H), dtype=np.float32)
    s = rng.standard_normal((B, H), dtype=np.float32)
    w = (rng.random((H, H), dtype=np.float32) - 0.5) * (2.0 / np.sqrt(H))
    got = kernel(f, s, w)
    want = ((s @ w.T) * f).sum(-1)
    err = np.abs(got - want)
    print("absmax-rel:", err.max() / np.abs(want).max())

